# revision 17
# baseline (speedup 1.0000x reference)
"""Trainium2 Bass kernel for nn_Blur (upfirdn2d 4x4 blur, pad=(2,1)).

Formulation: out[i,j] = sum_{p,q} Kf[p,q] * x[i+p-2, j+q-2]   (Kf = flip(kernel2d))

For each W-tap q (4 taps), the H-convolution is a banded 64x64 matrix
Aq[i,h] = Kf[h-i+2, q].  Tolerance is 2e-2, so x streams as a single bf16
(the {1,3,9}/64 blur weights have <=4 mantissa bits: every bf16 product is
exact in fp32; end-to-end error ~5e-3) -- half the HBM traffic of an
fp32-faithful hi/lo split.

W-taps are fused in PAIRS into the K=128 contraction: x rows live in
partitions 0-63 (16 images x 64 cols, tight 128B stride -- the PE rhs
fetcher needs the power-of-two group stride to stream at 1 col/cycle);
one SBUF->SBUF queue DMA writes the same rows shifted left one column
into partitions 64-127.  Then
  pair(2,3): lhsT=[A2^T;A3^T], rhs c=0..62, out j=c   (tap3 reads dup)
  pair(0,1): lhsT=[A0^T;A1^T], rhs c=0..61, out j=c+2 (tap1 reads dup)
plus two N=1/image boundary matmuls (tap2@j=63, tap1@j=1, K=64 on the x
rows).  PSUM per-element has_written semantics: the FIRST matmul per
column group carries the only start=True (clearing the has-written state
across its partition range); every later matmul uses start=False, which
stores where clear and accumulates where set.  Two column groups run
concurrently on disjoint PE columns (tile_position (0,0)/(0,64)).
Tensor cost: ~1020 streamed cols/batch vs 2016 for a 4-tap scheme.

Input DMAs are issued 3 batches ahead on the sync queue so out-DMA
copy-waits never starve input issue (a stall >1.5us demotes the PE
clock-boost (HAM) to 1.2 GHz, which never recovers at <3.4us of
sustained activity).  The fp32 PSUM result is copied to SBUF as bf16
(alternating vector/scalar), DMA'd back, and cast to f32 on the host.
HBM per core: 8.4 MB in + 8.4 MB out = the ~47us roofline.

Sharding: the 16*512 = 8192 independent (n,c) images are split into 8
contiguous slabs of 1024 images, one per NeuronCore (data-parallel).
"""

import ml_dtypes
import numpy as np

import concourse.bacc as bacc
import concourse.bass as bass
import concourse.mybir as mybir
import concourse.tile as tile
from concourse.bass_utils import run_bass_kernel_spmd

N_CORES = 8
IMG = 64                      # H = W
N_IMAGES = 16 * 512           # 8192
PER_CORE = N_IMAGES // N_CORES  # 1024
GROUP = 16                    # images per batch
N_BATCH = PER_CORE // GROUP   # 64
TILE_W = GROUP * IMG          # 1024 free cols, 128B image stride
DT = mybir.dt.float32
IN_DT = mybir.dt.bfloat16
NP_IN = ml_dtypes.bfloat16

LAST_RESULTS = None  # BassKernelResults of the most recent run (for test.py)


def _build_weights(kernel2d: np.ndarray) -> np.ndarray:
    """[128, 256] bf16: cols 0:64=[A2^T;A3^T], 64:128=[A0^T;A1^T],
    128:192=[A2^T;0] (tap2 boundary), 192:256=[A1^T;0] (tap1 boundary)."""
    kf = np.flip(np.asarray(kernel2d, dtype=np.float64), (0, 1))
    a = np.zeros((4, IMG, IMG), dtype=np.float64)
    for q in range(4):
        for i in range(IMG):
            for p in range(4):
                h = i + p - 2
                if 0 <= h < IMG:
                    a[q, i, h] = kf[p, q]
    wts = np.zeros((128, 256), dtype=NP_IN)
    wts[:IMG, 0:IMG] = a[2].T.astype(NP_IN)
    wts[IMG:, 0:IMG] = a[3].T.astype(NP_IN)
    wts[:IMG, IMG:128] = a[0].T.astype(NP_IN)
    wts[IMG:, IMG:128] = a[1].T.astype(NP_IN)
    wts[:IMG, 128:192] = a[2].T.astype(NP_IN)
    wts[:IMG, 192:256] = a[1].T.astype(NP_IN)
    return wts


def _bass_module() -> bass.Bass:
    nc = bacc.Bacc(
        "TRN2",
        target_bir_lowering=False,
        debug=False,
        num_devices=N_CORES,
    )
    x_d = nc.dram_tensor("x", [N_BATCH, IMG, TILE_W], IN_DT, kind="ExternalInput")
    w_d = nc.dram_tensor("wts", [128, 256], IN_DT, kind="ExternalInput")
    o_d = nc.dram_tensor("out", [N_BATCH, 128, 512], IN_DT, kind="ExternalOutput")

    with tile.TileContext(nc) as tc:
        with (
            tc.tile_pool(name="const", bufs=1) as cpool,
            tc.tile_pool(name="inp", bufs=8) as ipool,
            tc.tile_pool(name="outp", bufs=8) as opool,
            tc.tile_pool(name="psum", bufs=8, space="PSUM") as ppool,
        ):
            w_tile = cpool.tile([128, 256], IN_DT)
            nc.sync.dma_start(w_tile[:], w_d[:])

            # HAM warmup: the PE clock-gate holds 1.2 GHz until ~3.4us of
            # sustained matmul activity.  Burn that window on dummy matmuls
            # (zeroed operands, result never read) that overlap the first
            # input DMA, so the real matmuls start at full clock.
            dummy = cpool.tile([128, 512], IN_DT, tag="warm_sbuf")
            nc.gpsimd.memset(dummy[:], 0.0)
            warm_ps = ppool.tile([128, 512], DT, tag="ps")
            for _ in range(12):
                nc.tensor.matmul(
                    warm_ps[:], dummy[:, 0:128], dummy[:], start=True, stop=True
                )

            def issue_in(b):
                t = ipool.tile([128, TILE_W], IN_DT)
                nc.sync.dma_start(t[0:IMG, :], x_d[b])
                # shifted dup: partitions 64-127 = x rows one col left.
                # Per-image col 63 holds next-image garbage; no matmul
                # window ever reads it.
                nc.sync.dma_start(
                    t[IMG:128, 0 : TILE_W - 1], t[0:IMG, 1:TILE_W]
                )
                in_tiles[b] = t

            LOOKAHEAD = 3
            in_tiles = {}
            for b in range(LOOKAHEAD):
                issue_in(b)

            for b in range(N_BATCH):
                if b + LOOKAHEAD < N_BATCH:
                    issue_in(b + LOOKAHEAD)
                in_tile = in_tiles.pop(b)
                rhs3 = in_tile[:].rearrange("p (g w) -> p g w", w=IMG)

                ps = ppool.tile([128, 512], DT)
                out3s = [
                    ps[cg * IMG : (cg + 1) * IMG, :].rearrange(
                        "p (g w) -> p g w", w=IMG
                    )
                    for cg in range(2)
                ]
                gss = [slice(cg * 8, (cg + 1) * 8) for cg in range(2)]
                # tap2 at j=63 goes FIRST: its start=True initializes the
                # per-element has_written state for the whole column group.
                for cg in range(2):
                    nc.tensor.matmul(
                        out3s[cg][:, :, IMG - 1 : IMG],
                        w_tile[0:IMG, 128:192],
                        rhs3[0:IMG, gss[cg], IMG - 1 : IMG],
                        start=True,
                        stop=False,
                        tile_position=(0, cg * IMG),
                        skip_group_check=True,
                    )
                # pair(2,3): stores into cleared elements j=0..62
                for cg in range(2):
                    nc.tensor.matmul(
                        out3s[cg][:, :, 0 : IMG - 1],
                        w_tile[:, 0:IMG],
                        rhs3[:, gss[cg], 0 : IMG - 1],
                        start=False,
                        stop=False,
                        tile_position=(0, cg * IMG),
                        skip_group_check=True,
                    )
                # tap1 at j=1 accumulates
                for cg in range(2):
                    nc.tensor.matmul(
                        out3s[cg][:, :, 1:2],
                        w_tile[0:IMG, 192:256],
                        rhs3[0:IMG, gss[cg], 0:1],
                        start=False,
                        stop=False,
                        tile_position=(0, cg * IMG),
                        skip_group_check=True,
                    )
                # pair(0,1): accumulates into j=2..63
                for cg in range(2):
                    nc.tensor.matmul(
                        out3s[cg][:, :, 2:IMG],
                        w_tile[:, IMG:128],
                        rhs3[:, gss[cg], 0 : IMG - 2],
                        start=False,
                        stop=True,
                        tile_position=(0, cg * IMG),
                        skip_group_check=True,
                    )

                out_tile = opool.tile([128, 512], IN_DT)
                if b % 2 == 0:
                    nc.vector.tensor_copy(out_tile[:], ps[:])
                    nc.sync.dma_start(o_d[b], out_tile[:])
                else:
                    nc.scalar.copy(out_tile[:], ps[:])
                    nc.scalar.dma_start(o_d[b], out_tile[:])
    nc.compile()
    return nc


def _host_pack(x: np.ndarray) -> np.ndarray:
    """FULL x (8192,64,64) f32 -> [N_CORES, N_BATCH, 64, 1024] bf16.

    Partition dim = h; free dim = (g: 16 images, w: 64), tightly packed."""
    v = x.reshape(N_CORES, N_BATCH, GROUP, IMG, IMG).transpose(0, 1, 3, 2, 4)
    return np.ascontiguousarray(v.astype(NP_IN)).reshape(
        N_CORES, N_BATCH, IMG, TILE_W
    )


def _host_unpack(tiles: np.ndarray) -> np.ndarray:
    """[N_CORES, N_BATCH, 128, 512] bf16 -> (8192, 64, 64) f32.

    Partition dim = (cg, i); free dim = (g: 8, j); img = b*16 + cg*8 + g."""
    v = tiles.reshape(N_CORES, N_BATCH, 2, IMG, 8, IMG)
    v = v.transpose(0, 1, 2, 4, 3, 5)  # [core, b, cg, g, i, j]
    return v.reshape(N_IMAGES, IMG, IMG).astype(np.float32)


def kernel(x: np.ndarray, kernel: np.ndarray, _trace: bool = False) -> np.ndarray:
    global LAST_RESULTS
    x = np.ascontiguousarray(np.asarray(x, dtype=np.float32))
    n, c, h, w = x.shape
    assert (n, c, h, w) == (16, 512, 64, 64), x.shape

    shards = _host_pack(x.reshape(N_IMAGES, IMG, IMG))
    wts = _build_weights(kernel)
    in_maps = [{"x": shards[i], "wts": wts} for i in range(N_CORES)]

    nc = _bass_module()
    results = run_bass_kernel_spmd(
        nc, in_maps, core_ids=list(range(N_CORES)), trace=_trace
    )
    LAST_RESULTS = results

    tiles = np.stack([r["out"] for r in results.results])
    out = _host_unpack(tiles)
    return np.ascontiguousarray(out.reshape(n, c, h, w)).astype(np.float32)
